# revision 3
# baseline (speedup 1.0000x reference)
"""Trainium2 Bass kernel v4 for IntervalClusterTriplet (hard-mining triplet loss).

Math: loss = mean_i relu(sqrt(max_{j in cluster(i)} d2_ij)
                       - sqrt(min_{j not in cluster(i)} d2_ij) + 1)
with d2_ij = n_i + n_j - 2 e_i.e_j. Only max/min VALUES are needed; n_i is
added per-partition after the reduce.

v4: native DVE ops only (HW runs custom accum table-ops at ~half rate, so
v3's fused custom reduces lost to stock instructions). Every PSUM tile gets
n_j accumulated by a second matmul (ones_mat @ esq) riding the same bank, so
no broadcast-norm tile is needed anywhere:
 - direct region (cols 0:2048, 4-bank PSUM tile, holds the same-cluster
   diagonal): two plain tensor_reduce(min) around the diag block, and the
   diag gets TT-add(mask)+reduce for the hard-positive max / masked min.
 - staged region (cols 2048:8192, 6 groups of 1024 per chunk): ACT evacuates
   PSUM -> SBUF bf16, DVE folds groups with a bf16 TT-min chain (2x packed
   mode) and one final reduce.
GPSIMD computes esq and the scalar epilogue; ACT does sqrt. bf16 staging
costs ~0.4% noise on d2, far inside the 2e-2 tolerance.

Sharding: rows of the distance matrix across 8 cores (1024 rows each); each
core gets E^T rolled so its own 1024 columns come first (one SPMD program).
Per-core output is the partial loss sum; host adds and divides by N.
"""

import numpy as np

import concourse.bacc as bacc
import concourse.mybir as mybir
import concourse.tile as tile
from concourse.bass_utils import run_bass_kernel_spmd

C, S, D = 1024, 8, 128
N = C * S              # 8192 embeddings
CORES = 8
M = N // CORES         # 1024 rows per core
P = 128                # partitions (rows per chunk)
CH = M // P            # 8 chunks per core
TN = 512               # one PSUM bank
GW = 1024              # staged group width (2 banks)
DW = 2048              # direct region width (4 banks)
NSTG = 6               # staged groups per chunk (cols 2048..8192)
BIG = 1.0e30
F32 = mybir.dt.float32
F32R = mybir.dt.float32r
BF16 = mybir.dt.bfloat16
ALU = mybir.AluOpType
AX = mybir.AxisListType
ACT = mybir.ActivationFunctionType

_CACHE: dict = {}


def build_program(reps: int = 1, mode: str = "full"):
    nc = bacc.Bacc("TRN2", target_bir_lowering=False, debug=False)
    et_d = nc.dram_tensor("et", [D, N], F32R, kind="ExternalInput").ap()
    m8max_d = nc.dram_tensor("m8max", [P, GW], F32, kind="ExternalInput").ap()
    m8min_d = nc.dram_tensor("m8min", [P, GW], F32, kind="ExternalInput").ap()
    onesm_d = nc.dram_tensor("onesm", [P, P], F32R, kind="ExternalInput").ap()
    onesc_d = nc.dram_tensor("onesc", [P, 2], F32R, kind="ExternalInput").ap()
    out_d = nc.dram_tensor("out", [1, 1], F32, kind="ExternalOutput").ap()

    def body(tc, cet, cmask, cones, iterp, esqp, work, stg, scrb, small,
             pg0, pstg):
        # ---- input DMAs (16 column chunks of et spread across queues)
        et = cet.tile([D, N], F32R, tag="et")
        for c in range(16):
            nc.sync.dma_start(et[:, c * TN:(c + 1) * TN],
                              et_d[:, c * TN:(c + 1) * TN])
        m8max = cmask.tile([P, GW], F32, tag="m8max")
        nc.sync.dma_start(m8max, m8max_d)
        m8min = cmask.tile([P, GW], F32, tag="m8min")
        nc.sync.dma_start(m8min, m8min_d)
        ones_m = cones.tile([P, P], F32R, tag="ones_m")
        nc.sync.dma_start(ones_m, onesm_d)
        ones_c = cones.tile([P, 2], F32R, tag="ones_c")
        nc.sync.dma_start(ones_c, onesc_d)

        # ---- setup
        em2 = iterp.tile([D, M], F32R, tag="em2")     # -2 * own embeddings
        nc.vector.tensor_scalar_mul(em2, et[:, 0:M], -2.0)

        # esq on GPSIMD, low columns first (first consumers come first)
        esq = esqp.tile([D, N], F32R, tag="esq")
        for h in range(4):
            nc.gpsimd.tensor_mul(esq[:, h * DW:(h + 1) * DW],
                                 et[:, h * DW:(h + 1) * DW],
                                 et[:, h * DW:(h + 1) * DW])

        # own-row squared norms per chunk, packed via one strided ACT copy
        nmy = iterp.tile([P, CH], F32, tag="nmy")
        pm = pstg.tile([P, GW], F32, tag="pg")
        for m in range(CH):
            nc.tensor.matmul(pm[:, 2 * m:2 * m + 2],
                             lhsT=esq[:, m * P:(m + 1) * P],
                             rhs=ones_c, start=True, stop=True)
        nc.scalar.copy(nmy, pm[:, 0:2 * CH:2])

        losses = work.tile([P, CH], F32, tag="losses")

        # ---- main loop over 8 row chunks
        for m in range(CH):
            lhs = em2[:, m * P:(m + 1) * P]
            mc = small.tile([P, 4], F32, tag="mc")
            nc.gpsimd.memset(mc, 3.0e38)
            apm = small.tile([P, 1], F32, tag="apm")
            dcol = m * P          # diag block at cols dcol:dcol+128

            # direct region: cols 0:2048 in one 4-bank psum tile, n_j
            # accumulated by the ones-matmul on the same banks
            pd = pg0.tile([P, DW], F32, tag="pd")
            for t in range(4):
                nc.tensor.matmul(pd[:, t * TN:(t + 1) * TN], lhsT=lhs,
                                 rhs=et[:, t * TN:(t + 1) * TN],
                                 start=True, stop=False)
            for t in range(4):
                nc.tensor.matmul(pd[:, t * TN:(t + 1) * TN], lhsT=ones_m,
                                 rhs=esq[:, t * TN:(t + 1) * TN],
                                 start=False, stop=True)
            if mode not in ("mm", "nodirect"):
                if dcol > 0:
                    nc.vector.tensor_reduce(mc[:, 1:2], pd[:, 0:dcol],
                                            axis=AX.X, op=ALU.min)
                nc.vector.tensor_reduce(mc[:, 2:3], pd[:, dcol + P:DW],
                                        axis=AX.X, op=ALU.min)
                dgx = small.tile([P, P], F32, tag="dgx")
                nc.vector.tensor_add(dgx, pd[:, dcol:dcol + P],
                                     m8max[:, dcol:dcol + P])
                nc.vector.tensor_reduce(apm, dgx, axis=AX.X, op=ALU.max)
                dgn = small.tile([P, P], F32, tag="dgn")
                nc.vector.tensor_add(dgn, pd[:, dcol:dcol + P],
                                     m8min[:, dcol:dcol + P])
                nc.vector.tensor_reduce(mc[:, 0:1], dgn, axis=AX.X,
                                        op=ALU.min)

            # staged region: 6 groups of 1024, in pairs for LDW batching;
            # consumed by a bf16 TT-min chain + one final reduce
            chain = None
            for q in range(NSTG // 2):
                pga = pstg.tile([P, GW], F32, tag="pg")
                pgb = pstg.tile([P, GW], F32, tag="pg")
                ba = DW + (2 * q) * GW
                bb = ba + GW
                for h in range(2):
                    nc.tensor.matmul(pga[:, h * TN:(h + 1) * TN], lhsT=lhs,
                                     rhs=et[:, ba + h * TN:ba + (h + 1) * TN],
                                     start=True, stop=False)
                for h in range(2):
                    nc.tensor.matmul(pgb[:, h * TN:(h + 1) * TN], lhsT=lhs,
                                     rhs=et[:, bb + h * TN:bb + (h + 1) * TN],
                                     start=True, stop=False)
                for h in range(2):
                    nc.tensor.matmul(pga[:, h * TN:(h + 1) * TN],
                                     lhsT=ones_m,
                                     rhs=esq[:, ba + h * TN:ba + (h + 1) * TN],
                                     start=False, stop=True)
                for h in range(2):
                    nc.tensor.matmul(pgb[:, h * TN:(h + 1) * TN],
                                     lhsT=ones_m,
                                     rhs=esq[:, bb + h * TN:bb + (h + 1) * TN],
                                     start=False, stop=True)
                sga = stg.tile([P, GW], BF16, tag="sg")
                nc.scalar.copy(sga, pga)
                sgb = stg.tile([P, GW], BF16, tag="sg")
                nc.scalar.copy(sgb, pgb)
                if mode not in ("mm", "nopair"):
                    nxt = scrb.tile([P, GW], BF16, tag="ch")
                    if chain is None:
                        nc.vector.tensor_tensor(nxt, sga, sgb, op=ALU.min)
                    else:
                        mid = scrb.tile([P, GW], BF16, tag="ch")
                        nc.vector.tensor_tensor(mid, chain, sga, op=ALU.min)
                        nc.vector.tensor_tensor(nxt, mid, sgb, op=ALU.min)
                    chain = nxt
            if mode not in ("mm", "nopair", "nodirect"):
                nc.vector.tensor_reduce(mc[:, 3:4], chain, axis=AX.X,
                                        op=ALU.min)
            if mode in ("mm", "noepi", "nodirect", "nopair"):
                nc.gpsimd.memset(losses[:, m:m + 1], 0.0)
                continue

            # ---- epilogue for this chunk's 128 rows
            anm = small.tile([P, 1], F32, tag="anm")
            nc.vector.tensor_reduce(anm, mc, axis=AX.X, op=ALU.min)
            apsq = small.tile([P, 1], F32, tag="apsq")
            nc.gpsimd.tensor_scalar(apsq, apm, nmy[:, m:m + 1], 0.0,
                                    op0=ALU.add, op1=ALU.max)
            ansq = small.tile([P, 1], F32, tag="ansq")
            nc.gpsimd.tensor_scalar(ansq, anm, nmy[:, m:m + 1], 0.0,
                                    op0=ALU.add, op1=ALU.max)
            ap = small.tile([P, 1], F32, tag="ap")
            nc.scalar.activation(ap, apsq, ACT.Sqrt)
            an = small.tile([P, 1], F32, tag="an")
            nc.scalar.activation(an, ansq, ACT.Sqrt)
            dmar = small.tile([P, 1], F32, tag="dmar")
            nc.gpsimd.tensor_sub(dmar, ap, an)
            nc.gpsimd.tensor_scalar(losses[:, m:m + 1], dmar, 1.0, 0.0,
                                    op0=ALU.add, op1=ALU.max)

        # ---- final: sum losses over chunks, then over partitions
        lsum = work.tile([P, 1], F32R, tag="lsum")
        with nc.allow_low_precision(reason="f32r rounding of per-row loss ok"):
            nc.vector.tensor_reduce(lsum, losses, axis=AX.X, op=ALU.add)
        pf = pstg.tile([P, GW], F32, tag="pg")
        nc.tensor.matmul(pf[0:1, 0:2], lhsT=lsum, rhs=ones_c, start=True,
                         stop=True)
        outsb = work.tile([1, 1], F32, tag="outsb")
        nc.scalar.copy(outsb, pf[0:1, 0:1])
        nc.sync.dma_start(out_d, outsb)

    with tile.TileContext(nc) as tc:
        with (
            tc.tile_pool(name="cet", bufs=2) as cet,
            tc.tile_pool(name="cmask", bufs=1) as cmask,
            tc.tile_pool(name="cones", bufs=2) as cones,
            tc.tile_pool(name="iterp", bufs=2) as iterp,
            tc.tile_pool(name="esqp", bufs=2) as esqp,
            tc.tile_pool(name="work", bufs=1) as work,
            tc.tile_pool(name="stg", bufs=8) as stg,
            tc.tile_pool(name="scrb", bufs=3) as scrb,
            tc.tile_pool(name="small", bufs=3) as small,
            tc.tile_pool(name="pg0", bufs=1, space="PSUM") as pg0,
            tc.tile_pool(name="pstg", bufs=2, space="PSUM") as pstg,
        ):
            args = (tc, cet, cmask, cones, iterp, esqp, work, stg, scrb,
                    small, pg0, pstg)
            if reps == 1:
                body(*args)
            else:
                with tc.For_i(0, reps, 1):
                    body(*args)

    nc.compile()
    return nc


def make_in_maps(batch: np.ndarray):
    E = np.ascontiguousarray(batch.reshape(N, D).astype(np.float32, copy=False))
    ET = np.ascontiguousarray(E.T)
    idx = np.arange(P)
    same = (idx[:, None] // S) == (idx[None, :] // S)
    mmin = np.where(same, BIG, 0.0).astype(np.float32)   # exclude same-cluster
    mmax = np.where(same, 0.0, -BIG).astype(np.float32)  # keep same-cluster
    m8min = np.tile(mmin, (1, GW // P)).astype(np.float32)
    m8max = np.tile(mmax, (1, GW // P)).astype(np.float32)
    in_maps = []
    for r in range(CORES):
        et_r = np.ascontiguousarray(np.roll(ET, -r * M, axis=1))
        in_maps.append({"et": et_r, "m8max": m8max, "m8min": m8min,
                        "onesm": np.ones((P, P), np.float32),
                        "onesc": np.ones((P, 2), np.float32)})
    return in_maps


def kernel(batch: np.ndarray) -> np.ndarray:
    if "nc" not in _CACHE:
        _CACHE["nc"] = build_program(reps=1)
    nc = _CACHE["nc"]
    in_maps = make_in_maps(np.asarray(batch))
    res = run_bass_kernel_spmd(nc, in_maps, core_ids=list(range(CORES)))
    total = sum(float(res.results[r]["out"][0, 0]) for r in range(CORES))
    return np.float32(total / N)


# revision 4
# speedup vs baseline: 1.2497x; 1.2497x over previous
"""Trainium2 Bass kernel v3 for IntervalClusterTriplet (hard-mining triplet loss).

Math: loss = mean_i relu(sqrt(max_{j in cluster(i)} d2_ij)
                       - sqrt(min_{j not in cluster(i)} d2_ij) + 1)
with d2_ij = n_i + n_j - 2 e_i.e_j. Only max/min VALUES are needed; n_i is
added per-partition after the reduce.

Design: the per-core 1024x8192 distance volume must leave PSUM through DVE
(1.04 ns/elem/lane) or ACT (0.83); the work is split so PE, DVE and ACT are
all near-balanced. Three custom DVE table ops (registered at import) do the
heavy lifting, each a single streaming pass with a fused reduce:
  ANT_ADD_MIN_RED: accum = min(s0, min_k(in0[k] + in1[k]))
  ANT_ADD_MAX_RED: accum = max(s0, max_k(in0[k] + in1[k]))
  ANT_MIN_MIN_RED: accum = min(s0, min_k(min(in0[k], in1[k])))
 - direct region (cols 0:2048, incl. the same-cluster diagonal): DVE
   ADD_MIN_RED (psum + nb) straight from a 4-bank PSUM tile, where nb is a
   broadcast row of column norms built via a ones-matrix matmul. The diag
   block uses ADD_MAX_RED / ADD_MIN_RED against (mask + nb) combo tiles.
 - staged region (cols 2048:8192, 6 groups of 1024 per chunk): n_j is
   accumulated into PSUM by a second matmul (ones_mat @ esq), ACT evacuates
   PSUM -> SBUF bf16, DVE consumes PAIRS of groups with one MIN_MIN_RED.
GPSIMD computes most of esq, the mask combos, and the scalar epilogue; ACT
does sqrt. bf16 staging costs ~0.4% relative noise on d2, far inside the
tolerance.

Sharding: rows of the distance matrix across 8 cores (1024 rows each); each
core gets E^T rolled so its own 1024 columns come first (one SPMD program).
Per-core output is the partial loss sum; host adds and divides by N.
"""

import numpy as np

import concourse.bacc as bacc
import concourse.mybir as mybir
import concourse.tile as tile
from concourse.bass_utils import run_bass_kernel_spmd

from concourse.dve_spec import Spec, Src0, Src1, C0, minn, maxx, lower
from concourse.dve_uop import DveOpSpec
import concourse.dve_ops as dops

C, S, D = 1024, 8, 128
N = C * S              # 8192 embeddings
CORES = 8
M = N // CORES         # 1024 rows per core
P = 128                # partitions (rows per chunk)
CH = M // P            # 8 chunks per core
TN = 512               # one PSUM bank
GW = 1024              # staged group width (2 banks)
DW = 2048              # direct region width (4 banks)
NSTG = 6               # staged groups per chunk (cols 2048..8192)
BIG = 1.0e30
F32 = mybir.dt.float32
F32R = mybir.dt.float32r
BF16 = mybir.dt.bfloat16
ALU = mybir.AluOpType
AX = mybir.AxisListType
ACT = mybir.ActivationFunctionType

_CACHE: dict = {}


def _ref_red(body_fn, red_fn):
    def _r(in0, in1, c0, c1, c2):
        b = body_fn(np.asarray(in0, np.float32),
                    np.asarray(in1, np.float32)).astype(np.float32)
        acc = red_fn(c0, b.reshape(b.shape[0], -1), red_fn)
        return b, acc
    return _r


def _red_min(c0, b, _):
    return np.minimum(c0, b.min(axis=-1, keepdims=True))


def _red_max(c0, b, _):
    return np.maximum(c0, b.max(axis=-1, keepdims=True))


def _register_op(name, body, accum, body_fn, red_fn):
    """Register a custom DVE table op (idempotent across re-imports)."""
    for o in dops.OPS:
        if o.name == name:
            return o
    spec = Spec(body=body, accum=accum, accum_init=C0,
                reference=_ref_red(body_fn, red_fn))
    op = dops.DveOp(name, spec, subdim=False, uops_sha={})
    dops.OPS.append(op)
    dops.CUSTOM_DVE_SPECS[name] = spec
    dops._SUB_OPCODE_FOR_NAME[name] = dops._CUSTOM_DVE_ROW_BASE + len(dops.OPS) - 1
    for ver in ("v3", "v4"):
        s = DveOpSpec(name=name, opcode=dops.get_dve_sub_opcode(name),
                      uops=lower(spec, ver=ver), rd1_en=True)
        op.uops_sha[ver] = s.sha(ver)
    return op


ADD_MIN = _register_op("ANT_ADD_MIN_RED", Src0 + Src1, minn,
                       lambda a, b: a + b, _red_min)
ADD_MAX = _register_op("ANT_ADD_MAX_RED", Src0 + Src1, maxx,
                       lambda a, b: a + b, _red_max)
MIN_MIN = _register_op("ANT_MIN_MIN_RED", minn(Src0, Src1), minn,
                       lambda a, b: np.minimum(a, b), _red_min)


def _unify_act_tables():
    """Make Copy resolve to the same activation-table set as Sqrt
    (sqrt_and_others) so the body uses ONE set and the per-iteration
    InstLoadActFuncSet gets hoisted out of the For_i loop. Set positions
    (= act_func_set_id indices) are preserved; only membership shrinks."""
    if getattr(bacc, "_ant_act_tables_unified", False):
        return
    orig = bacc.get_activation_tables
    A = mybir.ActivationFunctionType

    def patched(arch):
        tables = orig(arch)
        for name, funcs in tables.items():
            if name != "sqrt_and_others" and isinstance(funcs, set):
                funcs.discard(A.Copy)
        return tables

    bacc.get_activation_tables = patched
    bacc._ant_act_tables_unified = True


_unify_act_tables()


def build_program(reps: int = 1, mode: str = "full"):
    nc = bacc.Bacc("TRN2", target_bir_lowering=False, debug=False)
    et_d = nc.dram_tensor("et", [D, N], F32R, kind="ExternalInput").ap()
    m8max_d = nc.dram_tensor("m8max", [P, GW], F32, kind="ExternalInput").ap()
    m8min_d = nc.dram_tensor("m8min", [P, GW], F32, kind="ExternalInput").ap()
    onesm_d = nc.dram_tensor("onesm", [P, P], F32R, kind="ExternalInput").ap()
    onesc_d = nc.dram_tensor("onesc", [P, 2], F32R, kind="ExternalInput").ap()
    out_d = nc.dram_tensor("out", [1, 1], F32, kind="ExternalOutput").ap()

    def body(tc, cet, cmask, cones, iterp, esqp, work, stg, small, pg0, pstg):
        # ---- input DMAs (16 column chunks of et spread across queues)
        et = cet.tile([D, N], F32R, tag="et")
        for c in range(16):
            nc.sync.dma_start(et[:, c * TN:(c + 1) * TN],
                              et_d[:, c * TN:(c + 1) * TN])
        m8max = cmask.tile([P, GW], F32, tag="m8max")
        nc.sync.dma_start(m8max, m8max_d)
        m8min = cmask.tile([P, GW], F32, tag="m8min")
        nc.sync.dma_start(m8min, m8min_d)
        ones_m = cones.tile([P, P], F32R, tag="ones_m")
        nc.sync.dma_start(ones_m, onesm_d)
        ones_c = cones.tile([P, 2], F32R, tag="ones_c")
        nc.sync.dma_start(ones_c, onesc_d)

        # ---- setup
        em2 = iterp.tile([D, M], F32R, tag="em2")     # -2 * own embeddings
        nc.vector.tensor_scalar_mul(em2, et[:, 0:M], -2.0)

        # esq: DVE does cols 0:2048 (gates nb early), GPSIMD the rest in the
        # order the staged acc-matmuls will need them (low cols first)
        esq = esqp.tile([D, N], F32R, tag="esq")
        nc.vector.tensor_mul(esq[:, 0:DW], et[:, 0:DW], et[:, 0:DW])
        nc.gpsimd.tensor_mul(esq[:, DW:2 * DW], et[:, DW:2 * DW],
                             et[:, DW:2 * DW])

        # nb: column norms broadcast across partitions for cols 0:2048
        nb = iterp.tile([P, DW], F32, tag="nb")
        for h in range(2):
            pn = pstg.tile([P, GW], F32, tag="pg")
            for t in range(2):
                sl = slice(h * GW + t * TN, h * GW + (t + 1) * TN)
                nc.tensor.matmul(pn[:, t * TN:(t + 1) * TN], lhsT=ones_m,
                                 rhs=esq[:, sl], start=True, stop=True)
            nc.scalar.copy(nb[:, h * GW:(h + 1) * GW], pn)

        # combos for the diagonal block (mask + nb over own 1024 cols)
        comax = iterp.tile([P, GW], F32, tag="comax")
        nc.gpsimd.tensor_add(comax, m8max, nb[:, 0:GW])
        comin = iterp.tile([P, GW], F32, tag="comin")
        nc.gpsimd.tensor_add(comin, m8min, nb[:, 0:GW])

        nc.gpsimd.tensor_mul(esq[:, 2 * DW:3 * DW], et[:, 2 * DW:3 * DW],
                             et[:, 2 * DW:3 * DW])
        nc.gpsimd.tensor_mul(esq[:, 3 * DW:4 * DW], et[:, 3 * DW:4 * DW],
                             et[:, 3 * DW:4 * DW])

        # own-row squared norms per chunk, packed via one strided ACT copy
        nmy = iterp.tile([P, CH], F32, tag="nmy")
        pm = pstg.tile([P, GW], F32, tag="pg")
        for m in range(CH):
            nc.tensor.matmul(pm[:, 2 * m:2 * m + 2],
                             lhsT=esq[:, m * P:(m + 1) * P],
                             rhs=ones_c, start=True, stop=True)
        nc.scalar.copy(nmy, pm[:, 0:2 * CH:2])

        losses = work.tile([P, CH], F32, tag="losses")
        dummy = work.tile([P, 1], BF16, tag="dummy")

        # ---- main loop over 8 row chunks
        for m in range(CH):
            lhs = em2[:, m * P:(m + 1) * P]
            mc = small.tile([P, 8], F32, tag="mc")
            nc.gpsimd.memset(mc, 3.0e38)
            apm = small.tile([P, 1], F32, tag="apm")
            dcol = m * P          # diag block at cols dcol:dcol+128

            # direct region: cols 0:2048 in one 4-bank psum tile
            pd = pg0.tile([P, DW], F32, tag="pd")
            for t in range(4):
                nc.tensor.matmul(pd[:, t * TN:(t + 1) * TN], lhsT=lhs,
                                 rhs=et[:, t * TN:(t + 1) * TN],
                                 start=True, stop=True)
            if mode not in ("mm", "nodirect"):
                # nb-only reductions first (no combo dependency), then the
                # diag-masked max/min against the combo tiles
                if dcol > 0:
                    nc.vector._custom_dve(
                        ADD_MIN, out=dummy.broadcast_to(pd[:, 0:dcol].shape),
                        in0=pd[:, 0:dcol], in1=nb[:, 0:dcol], s0=3.0e38,
                        accum_out=mc[:, 1:2])
                nc.vector._custom_dve(
                    ADD_MIN, out=dummy.broadcast_to(pd[:, dcol + P:DW].shape),
                    in0=pd[:, dcol + P:DW], in1=nb[:, dcol + P:DW],
                    s0=3.0e38, accum_out=mc[:, 2:3])
                nc.vector._custom_dve(
                    ADD_MAX, out=dummy.broadcast_to(pd[:, dcol:dcol + P].shape),
                    in0=pd[:, dcol:dcol + P], in1=comax[:, dcol:dcol + P],
                    s0=-3.0e38, accum_out=apm)
                nc.vector._custom_dve(
                    ADD_MIN, out=dummy.broadcast_to(pd[:, dcol:dcol + P].shape),
                    in0=pd[:, dcol:dcol + P], in1=comin[:, dcol:dcol + P],
                    s0=3.0e38, accum_out=mc[:, 0:1])

            # staged region: 6 groups of 1024, in pairs for LDW batching
            for q in range(NSTG // 2):
                pga = pstg.tile([P, GW], F32, tag="pg")
                pgb = pstg.tile([P, GW], F32, tag="pg")
                ba = DW + (2 * q) * GW
                bb = ba + GW
                for h in range(2):
                    nc.tensor.matmul(pga[:, h * TN:(h + 1) * TN], lhsT=lhs,
                                     rhs=et[:, ba + h * TN:ba + (h + 1) * TN],
                                     start=True, stop=False)
                for h in range(2):
                    nc.tensor.matmul(pgb[:, h * TN:(h + 1) * TN], lhsT=lhs,
                                     rhs=et[:, bb + h * TN:bb + (h + 1) * TN],
                                     start=True, stop=False)
                for h in range(2):
                    nc.tensor.matmul(pga[:, h * TN:(h + 1) * TN],
                                     lhsT=ones_m,
                                     rhs=esq[:, ba + h * TN:ba + (h + 1) * TN],
                                     start=False, stop=True)
                for h in range(2):
                    nc.tensor.matmul(pgb[:, h * TN:(h + 1) * TN],
                                     lhsT=ones_m,
                                     rhs=esq[:, bb + h * TN:bb + (h + 1) * TN],
                                     start=False, stop=True)
                sga = stg.tile([P, GW], BF16, tag="sg")
                nc.scalar.copy(sga, pga)
                sgb = stg.tile([P, GW], BF16, tag="sg")
                nc.scalar.copy(sgb, pgb)
                if mode not in ("mm", "nopair"):
                    nc.vector._custom_dve(
                        MIN_MIN, out=dummy.broadcast_to(sga.shape), in0=sga,
                        in1=sgb, s0=3.0e38, accum_out=mc[:, 3 + q:4 + q])
            if mode in ("mm", "noepi", "nodirect"):
                nc.gpsimd.memset(losses[:, m:m + 1], 0.0)
                continue

            # ---- epilogue for this chunk's 128 rows
            anm = small.tile([P, 1], F32, tag="anm")
            nc.vector.tensor_reduce(anm, mc, axis=AX.X, op=ALU.min)
            apsq = small.tile([P, 1], F32, tag="apsq")
            nc.gpsimd.tensor_scalar(apsq, apm, nmy[:, m:m + 1], 0.0,
                                    op0=ALU.add, op1=ALU.max)
            ansq = small.tile([P, 1], F32, tag="ansq")
            nc.gpsimd.tensor_scalar(ansq, anm, nmy[:, m:m + 1], 0.0,
                                    op0=ALU.add, op1=ALU.max)
            ap = small.tile([P, 1], F32, tag="ap")
            nc.scalar.activation(ap, apsq, ACT.Sqrt)
            an = small.tile([P, 1], F32, tag="an")
            nc.scalar.activation(an, ansq, ACT.Sqrt)
            dmar = small.tile([P, 1], F32, tag="dmar")
            nc.gpsimd.tensor_sub(dmar, ap, an)
            nc.gpsimd.tensor_scalar(losses[:, m:m + 1], dmar, 1.0, 0.0,
                                    op0=ALU.add, op1=ALU.max)

        # ---- final: sum losses over chunks, then over partitions
        lsum = work.tile([P, 1], F32R, tag="lsum")
        with nc.allow_low_precision(reason="f32r rounding of per-row loss ok"):
            nc.vector.tensor_reduce(lsum, losses, axis=AX.X, op=ALU.add)
        pf = pstg.tile([P, GW], F32, tag="pg")
        nc.tensor.matmul(pf[0:1, 0:2], lhsT=lsum, rhs=ones_c, start=True,
                         stop=True)
        outsb = work.tile([1, 1], F32, tag="outsb")
        nc.scalar.copy(outsb, pf[0:1, 0:1])
        nc.sync.dma_start(out_d, outsb)

    with tile.TileContext(nc) as tc:
        with (
            tc.tile_pool(name="cet", bufs=2) as cet,
            tc.tile_pool(name="cmask", bufs=1) as cmask,
            tc.tile_pool(name="cones", bufs=2) as cones,
            tc.tile_pool(name="iterp", bufs=2) as iterp,
            tc.tile_pool(name="esqp", bufs=2) as esqp,
            tc.tile_pool(name="work", bufs=1) as work,
            tc.tile_pool(name="stg", bufs=8) as stg,
            tc.tile_pool(name="small", bufs=3) as small,
            tc.tile_pool(name="pg0", bufs=1, space="PSUM") as pg0,
            tc.tile_pool(name="pstg", bufs=2, space="PSUM") as pstg,
        ):
            args = (tc, cet, cmask, cones, iterp, esqp, work, stg, small,
                    pg0, pstg)
            if reps == 1:
                body(*args)
            else:
                with tc.For_i(0, reps, 1):
                    body(*args)

    nc.compile()
    return nc


def make_in_maps(batch: np.ndarray):
    E = np.ascontiguousarray(batch.reshape(N, D).astype(np.float32, copy=False))
    ET = np.ascontiguousarray(E.T)
    idx = np.arange(P)
    same = (idx[:, None] // S) == (idx[None, :] // S)
    mmin = np.where(same, BIG, 0.0).astype(np.float32)   # exclude same-cluster
    mmax = np.where(same, 0.0, -BIG).astype(np.float32)  # keep same-cluster
    m8min = np.tile(mmin, (1, GW // P)).astype(np.float32)
    m8max = np.tile(mmax, (1, GW // P)).astype(np.float32)
    in_maps = []
    for r in range(CORES):
        et_r = np.ascontiguousarray(np.roll(ET, -r * M, axis=1))
        in_maps.append({"et": et_r, "m8max": m8max, "m8min": m8min,
                        "onesm": np.ones((P, P), np.float32),
                        "onesc": np.ones((P, 2), np.float32)})
    return in_maps


def kernel(batch: np.ndarray) -> np.ndarray:
    if "nc" not in _CACHE:
        _CACHE["nc"] = build_program(reps=1)
    nc = _CACHE["nc"]
    in_maps = make_in_maps(np.asarray(batch))
    res = run_bass_kernel_spmd(nc, in_maps, core_ids=list(range(CORES)))
    total = sum(float(res.results[r]["out"][0, 0]) for r in range(CORES))
    return np.float32(total / N)


# revision 5
# speedup vs baseline: 1.3555x; 1.0846x over previous
"""Trainium2 Bass kernel v7 for IntervalClusterTriplet (hard-mining triplet loss).

Math: loss = mean_i relu(sqrt(max_{j in cluster(i)} d2_ij)
                       - sqrt(min_{j not in cluster(i)} d2_ij) + 1)
with d2_ij = n_i + n_j - 2 e_i.e_j. Only max/min VALUES are needed; n_i is
added per-partition after the reduce.

Design: the per-core 1024x8192 distance volume must leave PSUM through DVE
(1.04 ns/elem/lane) or ACT (0.83); the work is split so PE, DVE and ACT are
all near-balanced. Three custom DVE table ops (registered at import) do the
heavy lifting, each a single streaming pass with a fused reduce:
  ANT_ADD_MIN_RED: accum = min(s0, min_k(in0[k] + in1[k]))
  ANT_ADD_MAX_RED: accum = max(s0, max_k(in0[k] + in1[k]))
  ANT_MIN_MIN_RED: accum = min(s0, min_k(min(in0[k], in1[k])))
 - direct region (cols 0:2048, incl. the same-cluster diagonal): DVE
   ADD_MIN_RED (psum + nb) straight from a 4-bank PSUM tile, where nb is a
   broadcast row of column norms built via a ones-matrix matmul. The diag
   block uses ADD_MAX_RED / ADD_MIN_RED against (mask + nb) combo tiles.
 - staged region (cols 2048:8192, 6 groups of 1024 per chunk): n_j is
   accumulated into PSUM by a second matmul (ones_mat @ esq), ACT evacuates
   PSUM -> SBUF bf16, DVE consumes PAIRS of groups with one MIN_MIN_RED.
GPSIMD computes most of esq, the mask combos, and the scalar epilogue; ACT
does sqrt. bf16 staging costs ~0.4% relative noise on d2, far inside the
tolerance.

Sharding: rows of the distance matrix across 8 cores (1024 rows each); each
core gets E^T rolled so its own 1024 columns come first (one SPMD program).
Per-core output is the partial loss sum; host adds and divides by N.
"""

import numpy as np

import concourse.bacc as bacc
import concourse.mybir as mybir
import concourse.tile as tile
from concourse.bass_utils import run_bass_kernel_spmd

from concourse.dve_spec import Spec, Src0, Src1, C0, minn, maxx, lower
from concourse.dve_uop import DveOpSpec
import concourse.dve_ops as dops

C, S, D = 1024, 8, 128
N = C * S              # 8192 embeddings
CORES = 8
M = N // CORES         # 1024 rows per core
P = 128                # partitions (rows per chunk)
CH = M // P            # 8 chunks per core
TN = 512               # one PSUM bank
GW = 1024              # staged group width (2 banks)
DW = 1024              # direct region width (2 banks)
NSTG = 7               # staged groups per chunk (cols 1024..8192)
BIG = 1.0e30
F32 = mybir.dt.float32
F32R = mybir.dt.float32r
BF16 = mybir.dt.bfloat16
ALU = mybir.AluOpType
AX = mybir.AxisListType
ACT = mybir.ActivationFunctionType

_CACHE: dict = {}


def _ref_red(body_fn, red_fn):
    def _r(in0, in1, c0, c1, c2):
        b = body_fn(np.asarray(in0, np.float32),
                    np.asarray(in1, np.float32)).astype(np.float32)
        acc = red_fn(c0, b.reshape(b.shape[0], -1), red_fn)
        return b, acc
    return _r


def _red_min(c0, b, _):
    return np.minimum(c0, b.min(axis=-1, keepdims=True))


def _red_max(c0, b, _):
    return np.maximum(c0, b.max(axis=-1, keepdims=True))


def _register_op(name, body, accum, body_fn, red_fn):
    """Register a custom DVE table op (idempotent across re-imports)."""
    for o in dops.OPS:
        if o.name == name:
            return o
    spec = Spec(body=body, accum=accum, accum_init=C0,
                reference=_ref_red(body_fn, red_fn))
    op = dops.DveOp(name, spec, subdim=False, uops_sha={})
    dops.OPS.append(op)
    dops.CUSTOM_DVE_SPECS[name] = spec
    dops._SUB_OPCODE_FOR_NAME[name] = dops._CUSTOM_DVE_ROW_BASE + len(dops.OPS) - 1
    for ver in ("v3", "v4"):
        s = DveOpSpec(name=name, opcode=dops.get_dve_sub_opcode(name),
                      uops=lower(spec, ver=ver), rd1_en=True)
        op.uops_sha[ver] = s.sha(ver)
    return op


ADD_MIN = _register_op("ANT_ADD_MIN_RED", Src0 + Src1, minn,
                       lambda a, b: a + b, _red_min)
ADD_MAX = _register_op("ANT_ADD_MAX_RED", Src0 + Src1, maxx,
                       lambda a, b: a + b, _red_max)
MIN_MIN = _register_op("ANT_MIN_MIN_RED", minn(Src0, Src1), minn,
                       lambda a, b: np.minimum(a, b), _red_min)


def _unify_act_tables():
    """Make Copy resolve to the same activation-table set as Sqrt
    (sqrt_and_others) so the body uses ONE set and the per-iteration
    InstLoadActFuncSet gets hoisted out of the For_i loop. Set positions
    (= act_func_set_id indices) are preserved; only membership shrinks."""
    if getattr(bacc, "_ant_act_tables_unified", False):
        return
    orig = bacc.get_activation_tables
    A = mybir.ActivationFunctionType

    def patched(arch):
        tables = orig(arch)
        for name, funcs in tables.items():
            if name != "sqrt_and_others" and isinstance(funcs, set):
                funcs.discard(A.Copy)
        return tables

    bacc.get_activation_tables = patched
    bacc._ant_act_tables_unified = True


_unify_act_tables()


def build_program(reps: int = 1, mode: str = "full"):
    nc = bacc.Bacc("TRN2", target_bir_lowering=False, debug=False)
    et_d = nc.dram_tensor("et", [D, N], F32R, kind="ExternalInput").ap()
    m8max_d = nc.dram_tensor("m8max", [P, GW], F32, kind="ExternalInput").ap()
    m8min_d = nc.dram_tensor("m8min", [P, GW], F32, kind="ExternalInput").ap()
    onesm_d = nc.dram_tensor("onesm", [P, P], F32R, kind="ExternalInput").ap()
    onesc_d = nc.dram_tensor("onesc", [P, 2], F32R, kind="ExternalInput").ap()
    out_d = nc.dram_tensor("out", [1, 1], F32, kind="ExternalOutput").ap()

    def body(tc, cet, cmask, cones, iterp, esqp, work, stg, small, pg0, pstg):
        # ---- input DMAs (16 column chunks of et spread across queues)
        et = cet.tile([D, N], F32R, tag="et")
        for c in range(16):
            nc.sync.dma_start(et[:, c * TN:(c + 1) * TN],
                              et_d[:, c * TN:(c + 1) * TN])
        m8max = cmask.tile([P, GW], F32, tag="m8max")
        nc.sync.dma_start(m8max, m8max_d)
        m8min = cmask.tile([P, GW], F32, tag="m8min")
        nc.sync.dma_start(m8min, m8min_d)
        ones_m = cones.tile([P, P], F32R, tag="ones_m")
        nc.sync.dma_start(ones_m, onesm_d)
        ones_c = cones.tile([P, 2], F32R, tag="ones_c")
        nc.sync.dma_start(ones_c, onesc_d)

        # ---- setup
        em2 = iterp.tile([D, M], F32R, tag="em2")     # -2 * own embeddings
        nc.vector.tensor_scalar_mul(em2, et[:, 0:M], -2.0)

        # esq: DVE does cols 0:2048 (gates nb early), GPSIMD the rest in the
        # order the staged acc-matmuls will need them (low cols first)
        esq = esqp.tile([D, N], F32R, tag="esq")
        nc.vector.tensor_mul(esq[:, 0:2048], et[:, 0:2048], et[:, 0:2048])
        nc.gpsimd.tensor_mul(esq[:, 2048:4096], et[:, 2048:4096],
                             et[:, 2048:4096])

        # nb: column norms broadcast across partitions for cols 0:2048
        nb = iterp.tile([P, DW], F32, tag="nb")
        pn = pstg.tile([P, GW], F32, tag="pg")
        for t in range(2):
            nc.tensor.matmul(pn[:, t * TN:(t + 1) * TN], lhsT=ones_m,
                             rhs=esq[:, t * TN:(t + 1) * TN],
                             start=True, stop=True)
        nc.scalar.copy(nb, pn)

        # combos for the diagonal block (mask + nb over own 1024 cols)
        comax = iterp.tile([P, GW], F32, tag="comax")
        nc.gpsimd.tensor_add(comax, m8max, nb[:, 0:GW])
        comin = iterp.tile([P, GW], F32, tag="comin")
        nc.gpsimd.tensor_add(comin, m8min, nb[:, 0:GW])

        nc.gpsimd.tensor_mul(esq[:, 4096:6144], et[:, 4096:6144],
                             et[:, 4096:6144])
        nc.gpsimd.tensor_mul(esq[:, 6144:8192], et[:, 6144:8192],
                             et[:, 6144:8192])

        # own-row squared norms per chunk, packed via one strided ACT copy
        nmy = iterp.tile([P, CH], F32, tag="nmy")
        pm = pstg.tile([P, GW], F32, tag="pg")
        for m in range(CH):
            nc.tensor.matmul(pm[:, 2 * m:2 * m + 2],
                             lhsT=esq[:, m * P:(m + 1) * P],
                             rhs=ones_c, start=True, stop=True)
        nc.scalar.copy(nmy, pm[:, 0:2 * CH:2])

        losses = work.tile([P, CH], F32, tag="losses")
        dummy = work.tile([P, 1], BF16, tag="dummy")

        # ---- main loop over 8 row chunks
        for m in range(CH):
            lhs = em2[:, m * P:(m + 1) * P]
            mc = small.tile([P, 8], F32, tag="mc")
            nc.gpsimd.memset(mc, 3.0e38)
            apm = small.tile([P, 1], F32, tag="apm")
            dcol = m * P          # diag block at cols dcol:dcol+128

            # direct region: cols 0:2048 in one 4-bank psum tile
            pd = pg0.tile([P, DW], F32, tag="pd")
            for t in range(2):
                nc.tensor.matmul(pd[:, t * TN:(t + 1) * TN], lhsT=lhs,
                                 rhs=et[:, t * TN:(t + 1) * TN],
                                 start=True, stop=True)
            if mode not in ("mm", "nodirect"):
                # nb-only reductions first (no combo dependency), then the
                # diag-masked max/min against the combo tiles
                if dcol > 0:
                    nc.vector._custom_dve(
                        ADD_MIN, out=dummy.broadcast_to(pd[:, 0:dcol].shape),
                        in0=pd[:, 0:dcol], in1=nb[:, 0:dcol], s0=3.0e38,
                        accum_out=mc[:, 1:2])
                if dcol + P < DW:
                    nc.vector._custom_dve(
                        ADD_MIN,
                        out=dummy.broadcast_to(pd[:, dcol + P:DW].shape),
                        in0=pd[:, dcol + P:DW], in1=nb[:, dcol + P:DW],
                        s0=3.0e38, accum_out=mc[:, 2:3])
                nc.vector._custom_dve(
                    ADD_MAX, out=dummy.broadcast_to(pd[:, dcol:dcol + P].shape),
                    in0=pd[:, dcol:dcol + P], in1=comax[:, dcol:dcol + P],
                    s0=-3.0e38, accum_out=apm)
                nc.vector._custom_dve(
                    ADD_MIN, out=dummy.broadcast_to(pd[:, dcol:dcol + P].shape),
                    in0=pd[:, dcol:dcol + P], in1=comin[:, dcol:dcol + P],
                    s0=3.0e38, accum_out=mc[:, 0:1])

            # staged region: 6 groups of 1024, in pairs for LDW batching
            for q in range(4):
                pga = pstg.tile([P, GW], F32, tag="pg")
                ba = DW + (2 * q) * GW
                bb = ba + GW
                solo = q == 3
                pgb = None if solo else pstg.tile([P, GW], F32, tag="pg")
                for h in range(2):
                    nc.tensor.matmul(pga[:, h * TN:(h + 1) * TN], lhsT=lhs,
                                     rhs=et[:, ba + h * TN:ba + (h + 1) * TN],
                                     start=True, stop=False)
                if not solo:
                    for h in range(2):
                        nc.tensor.matmul(pgb[:, h * TN:(h + 1) * TN],
                                         lhsT=lhs,
                                         rhs=et[:, bb + h * TN:bb + (h + 1) * TN],
                                         start=True, stop=False)
                for h in range(2):
                    nc.tensor.matmul(pga[:, h * TN:(h + 1) * TN],
                                     lhsT=ones_m,
                                     rhs=esq[:, ba + h * TN:ba + (h + 1) * TN],
                                     start=False, stop=True)
                if not solo:
                    for h in range(2):
                        nc.tensor.matmul(pgb[:, h * TN:(h + 1) * TN],
                                         lhsT=ones_m,
                                         rhs=esq[:, bb + h * TN:bb + (h + 1) * TN],
                                         start=False, stop=True)
                sga = stg.tile([P, GW], BF16, tag="sg")
                nc.scalar.copy(sga, pga)
                if not solo:
                    sgb = stg.tile([P, GW], BF16, tag="sg")
                    nc.scalar.copy(sgb, pgb)
                if mode not in ("mm", "nopair"):
                    if solo:
                        nc.vector._custom_dve(
                            MIN_MIN,
                            out=dummy.broadcast_to(sga[:, 0:TN].shape),
                            in0=sga[:, 0:TN], in1=sga[:, TN:GW], s0=3.0e38,
                            accum_out=mc[:, 3 + q:4 + q])
                    else:
                        nc.vector._custom_dve(
                            MIN_MIN, out=dummy.broadcast_to(sga.shape),
                            in0=sga, in1=sgb, s0=3.0e38,
                            accum_out=mc[:, 3 + q:4 + q])
            if mode in ("mm", "noepi", "nodirect"):
                nc.gpsimd.memset(losses[:, m:m + 1], 0.0)
                continue

            # ---- epilogue for this chunk's 128 rows
            anm = small.tile([P, 1], F32, tag="anm")
            nc.vector.tensor_reduce(anm, mc, axis=AX.X, op=ALU.min)
            apsq = small.tile([P, 1], F32, tag="apsq")
            nc.gpsimd.tensor_scalar(apsq, apm, nmy[:, m:m + 1], 0.0,
                                    op0=ALU.add, op1=ALU.max)
            ansq = small.tile([P, 1], F32, tag="ansq")
            nc.gpsimd.tensor_scalar(ansq, anm, nmy[:, m:m + 1], 0.0,
                                    op0=ALU.add, op1=ALU.max)
            ap = small.tile([P, 1], F32, tag="ap")
            nc.scalar.activation(ap, apsq, ACT.Sqrt)
            an = small.tile([P, 1], F32, tag="an")
            nc.scalar.activation(an, ansq, ACT.Sqrt)
            dmar = small.tile([P, 1], F32, tag="dmar")
            nc.gpsimd.tensor_sub(dmar, ap, an)
            nc.gpsimd.tensor_scalar(losses[:, m:m + 1], dmar, 1.0, 0.0,
                                    op0=ALU.add, op1=ALU.max)

        # ---- final: sum losses over chunks, then over partitions
        lsum = work.tile([P, 1], F32R, tag="lsum")
        with nc.allow_low_precision(reason="f32r rounding of per-row loss ok"):
            nc.vector.tensor_reduce(lsum, losses, axis=AX.X, op=ALU.add)
        pf = pstg.tile([P, GW], F32, tag="pg")
        nc.tensor.matmul(pf[0:1, 0:2], lhsT=lsum, rhs=ones_c, start=True,
                         stop=True)
        outsb = work.tile([1, 1], F32, tag="outsb")
        nc.scalar.copy(outsb, pf[0:1, 0:1])
        nc.sync.dma_start(out_d, outsb)

    with tile.TileContext(nc) as tc:
        with (
            tc.tile_pool(name="cet", bufs=2) as cet,
            tc.tile_pool(name="cmask", bufs=1) as cmask,
            tc.tile_pool(name="cones", bufs=2) as cones,
            tc.tile_pool(name="iterp", bufs=2) as iterp,
            tc.tile_pool(name="esqp", bufs=2) as esqp,
            tc.tile_pool(name="work", bufs=1) as work,
            tc.tile_pool(name="stg", bufs=9) as stg,
            tc.tile_pool(name="small", bufs=3) as small,
            tc.tile_pool(name="pg0", bufs=1, space="PSUM") as pg0,
            tc.tile_pool(name="pstg", bufs=3, space="PSUM") as pstg,
        ):
            args = (tc, cet, cmask, cones, iterp, esqp, work, stg, small,
                    pg0, pstg)
            if reps == 1:
                body(*args)
            else:
                with tc.For_i(0, reps, 1):
                    body(*args)

    nc.compile()
    return nc


def make_in_maps(batch: np.ndarray):
    E = np.ascontiguousarray(batch.reshape(N, D).astype(np.float32, copy=False))
    ET = np.ascontiguousarray(E.T)
    idx = np.arange(P)
    same = (idx[:, None] // S) == (idx[None, :] // S)
    mmin = np.where(same, BIG, 0.0).astype(np.float32)   # exclude same-cluster
    mmax = np.where(same, 0.0, -BIG).astype(np.float32)  # keep same-cluster
    m8min = np.tile(mmin, (1, GW // P)).astype(np.float32)
    m8max = np.tile(mmax, (1, GW // P)).astype(np.float32)
    in_maps = []
    for r in range(CORES):
        et_r = np.ascontiguousarray(np.roll(ET, -r * M, axis=1))
        in_maps.append({"et": et_r, "m8max": m8max, "m8min": m8min,
                        "onesm": np.ones((P, P), np.float32),
                        "onesc": np.ones((P, 2), np.float32)})
    return in_maps


def kernel(batch: np.ndarray) -> np.ndarray:
    if "nc" not in _CACHE:
        _CACHE["nc"] = build_program(reps=1)
    nc = _CACHE["nc"]
    in_maps = make_in_maps(np.asarray(batch))
    res = run_bass_kernel_spmd(nc, in_maps, core_ids=list(range(CORES)))
    total = sum(float(res.results[r]["out"][0, 0]) for r in range(CORES))
    return np.float32(total / N)
